# revision 47
# baseline (speedup 1.0000x reference)
"""DNA-structure attention Trainium2 kernel (8-core SPMD), v3.

Reference computation (per batch b):
    qkv = x @ qkv_w.T + qkv_b ; split to q,k,v [H=16 heads, d=64]
    s   = q @ k.T / 8 + dna_bias ; causal mask ; p = softmax(s)
    o   = p @ v ; y = concat_heads(o) @ out_w.T + out_b

Sharding: 8 cores = 4 batches x 2 head-groups (8 heads each).
Each core computes its batch's partial output y_partial = o_g @ out_w[:, cols_g].T;
host sums the two partials per batch and adds out_b.

v3 changes over v2 (v2 = 300.4us -> v3 = ~269us):
  - Paired score matmuls: head 2i lives at PE rows 0-63, head 2i+1 at rows
    64-127 (the layout already guaranteed this). Emitting the two heads'
    score MMs back-to-back makes the PE run them CONCURRENTLY as 64-row
    tiles (microbenched: exactly 2.0x) -- the d=64 contraction no longer
    wastes half the array. Critical detail: both heads' scores for one
    kc-block share ONE 2-bank PSUM tile (h0 at bank 0, h1 at bank 1) so a
    SINGLE exp covers both -- with per-head tiles the next round's pair
    serializes on two different exp semaphores (measured, kills the 2x).
  - Normalization 1/d via ScalarE copy + DVE reciprocal_approx_fast
    (18-bit ok; 51-ULP) instead of ScalarE exp(-ln d): frees ~35us of
    ScalarE for the softmax exps. The custom DVE op needs same-base-
    partition SBUF operands (cross-base or PSUM input = garbage/hang).
  - attn@v matmuls drained BEFORE each kp's score MMs: the first score MM
    waits the previous kp's exp (single-buffered PSUM), and work queued
    ahead of it covers the wait (in-order PE queue head-blocks otherwise).
  - Startup: host pre-swizzles weights to the SBUF layout so each loads
    with a handful of descriptors (one contiguous descriptor = ONE DMA
    engine at ~37GB/s; splitting is what buys parallelism, but each
    dma_start costs ~0.6us of issue time on its queue -- balance). 16
    dummy matmuls pre-warm the PE HAM clock gate (1.2 -> 2.4GHz) during
    the DMA wait. First real matmul at ~11us (was ~21).
  - y written block-contiguous (host untransposes): linear 128KB writes
    instead of 128-segment strided scatters.
  - Phase-4 tail: last 4 out-proj units held out of the interleave and
    emitted between attn(3)'s main stream and its deferred final
    avs+normalizes, covering the last exp->mult->av latency bubble.
"""

import sys

if "/opt/trn_rl_repo" not in sys.path:
    sys.path.insert(0, "/opt/trn_rl_repo")

import numpy as np

import concourse.bass as bass
import concourse.mybir as mybir
import concourse.tile as tile
from concourse import bacc
from concourse.bass_utils import run_bass_kernel_spmd

# The axon NTFF-profiling hook lives in trn_agent_boot in this container but
# concourse expects it at antenv.axon_hooks (absent). Register a shim module
# so run_bass_kernel_spmd(trace=True) can capture real HW timings.
if "antenv.axon_hooks" not in sys.modules:
    import types

    def _get_axon_ntff_profile_hook(_cache=[]):
        if not _cache:
            try:
                from trn_agent_boot.trn_boot import _ntff_profile_via_ctypes
                _cache.append(
                    _ntff_profile_via_ctypes("/opt/axon/libaxon_pjrt.so"))
            except Exception:
                _cache.append(None)
        return _cache[0]

    _m = types.ModuleType("antenv.axon_hooks")
    _m.get_axon_ntff_profile_hook = _get_axon_ntff_profile_hook
    sys.modules["antenv.axon_hooks"] = _m

B, T, DIM = 4, 2048, 1024
HEADS = 16
HD = 64  # head dim
N_CORES = 8
HPC = 8            # heads per core
CPC = HPC * HD     # channel slice per core (512)
QC = 512           # query chunk
KC = 128           # key chunk
N_QC = T // QC     # 4
N_KC = T // KC     # 16
P = 128

F32 = mybir.dt.float32
BF16 = mybir.dt.bfloat16

NCC = DIM // P    # 8 contraction chunks for qkv projection
NQD = CPC // P    # 4 dim-chunks of Q/K


def _qc_pairs(qc):
    """Ragged causal plan for query chunk qc: list of pairs, each pair a
    list of (kc, j, cols) with j = 128-query offset into the chunk."""
    n_kc = 4 * (qc + 1)
    blocks = []
    for kc in range(n_kc):
        j = max(0, kc - 4 * qc)
        blocks.append((kc, j, QC - j * KC))
    return [blocks[i:i + 2] for i in range(0, n_kc, 2)]


def _qc_cols(qc):
    return sum(c for pr in _qc_pairs(qc) for (_, _, c) in pr)


E_STRIDE = _qc_cols(3)  # 7424, worst-case packed width


def build_program():
    nc = bacc.Bacc("TRN2", target_bir_lowering=False, debug=False,
                   num_devices=N_CORES)

    # weights/biases come in host-preswizzled to the SBUF layout
    # [128 partitions, chunk-major cols] so each loads as ONE DMA
    # descriptor (a "(c p) m -> p c m" rearrange on the DRAM side gets
    # split into NCC separate DIRECT2D issues at ~0.6us each, serialized
    # on the issuing sequencer -- that was ~15us of the v3 startup).
    x_t = nc.declare_dram_parameter("x_t", [DIM, T], BF16, isOutput=False)
    wq_t = nc.declare_dram_parameter("wq_t", [P, NCC * CPC], BF16,
                                     isOutput=False)
    wk_t = nc.declare_dram_parameter("wk_t", [P, NCC * CPC], BF16,
                                     isOutput=False)
    wv_t = nc.declare_dram_parameter("wv_t", [P, NCC * CPC], BF16,
                                     isOutput=False)
    bq = nc.declare_dram_parameter("bq", [P, NQD], F32, isOutput=False)
    bk = nc.declare_dram_parameter("bk", [P, NQD], F32, isOutput=False)
    bv = nc.declare_dram_parameter("bv", [P, CPC], F32, isOutput=False)
    wo_t = nc.declare_dram_parameter("wo_t", [P, NQD * DIM], BF16,
                                     isOutput=False)
    e_pk = nc.declare_dram_parameter("e_pk", [P, N_QC * E_STRIDE], BF16,
                                     isOutput=False)
    # y partials in bf16: halves the output DMA and the two partials are
    # summed in fp32 on the host (absmax error budget has 2x headroom).
    # Block-contiguous layout [qc*4+m, half, 128, 512]: each out-proj
    # result block is one LINEAR 128KB DMA write (the [T, DIM] layout made
    # every write a 128-segment strided scatter that drained ~5us after
    # the last matmul); the host untransposes.
    y = nc.declare_dram_parameter("y", [N_QC * 4 * 2 * P, QC], BF16,
                                  isOutput=True)

    with tile.TileContext(nc) as tc:
        with (
            tc.tile_pool(name="persist", bufs=1) as persist,
            tc.tile_pool(name="wts", bufs=1) as wts,
            tc.tile_pool(name="xw", bufs=2) as xw,
            tc.tile_pool(name="ebuf", bufs=1) as ebuf,
            tc.tile_pool(name="otile", bufs=4) as otile,
            tc.tile_pool(name="wkp", bufs=8) as wkp,
            tc.tile_pool(name="wko", bufs=3) as wko,
            tc.tile_pool(name="wkn", bufs=2) as wkn,
            tc.tile_pool(name="psA", bufs=2, space="PSUM") as psA,
            tc.tile_pool(name="psS", bufs=2, space="PSUM") as psS,
            tc.tile_pool(name="psO", bufs=2, space="PSUM") as psO,
        ):
            # ---------------- persistent SBUF ----------------
            qt_buf = persist.tile([P, NQD, T], BF16)   # Q.T, dims-major
            kt_buf = persist.tile([P, NQD, T], BF16)   # K.T, same layout
            # V' per (key chunk, head): cols 0:64 V, cols 64:128 ones
            vp_buf = persist.tile([P, N_KC, HPC, P], BF16)
            bq_sb = persist.tile([P, NQD], F32)
            bk_sb = persist.tile([P, NQD], F32)
            bv_sb = persist.tile([P, CPC], F32)
            wq_sb = wts.tile([P, NCC, CPC], BF16)
            wk_sb = wts.tile([P, NCC, CPC], BF16)
            wv_sb = wts.tile([P, NCC, CPC], BF16)
            wo_sb = wts.tile([P, NQD, DIM], BF16)      # out_w.T slice

            x_t3 = x_t.rearrange("(c p) t -> p c t", p=P)

            QW = 2 * QC               # projection window (1024 tokens)

            def stage_xts(qp, split=False):
                tw = slice(qp * QW, (qp + 1) * QW)
                xt = xw.tile([P, NCC, QW], BF16, tag="xt")
                if split:
                    # first 512 tokens in chunk-pair pieces: the first
                    # matmul only needs dim-chunk 0, and smaller descriptors
                    # spread over more DMA engines
                    for c in range(0, NCC, 2):
                        nc.scalar.dma_start(
                            xt[:, c:c + 2, 0:QC],
                            x_t3[:, c:c + 2, tw.start:tw.start + QC])
                else:
                    nc.sync.dma_start(xt[:], x_t3[:, :, tw])
                return xt

            # E tiles: ping-pong two buffers across qc (qc0/2 share one,
            # qc1/3 the other, sized to the larger user)
            e_tiles = {}
            E_TAG_COLS = {0: _qc_cols(2), 1: _qc_cols(3)}

            def stage_e(qc, eng=None):
                eng = eng or nc.gpsimd
                t_ = ebuf.tile([P, E_TAG_COLS[qc % 2]], BF16, tag=f"e{qc % 2}")
                e_tiles[qc] = t_
                cols = _qc_cols(qc)
                # split into <=2048-col chunks: one contiguous descriptor
                # runs on a SINGLE DMA engine (~37GB/s), so chunking is what
                # buys transfer parallelism
                off = 0
                while off < cols:
                    n = min(2048, cols - off)
                    eng.dma_start(
                        t_[:, off:off + n],
                        e_pk[:, qc * E_STRIDE + off: qc * E_STRIDE + off + n])
                    off += n

            # Startup DMA choreography: one fat DMA per tensor, spread
            # across the engine queues that can issue DMAs (sync, scalar,
            # gpsimd) so the transfers run concurrently (v2 serialized
            # wq -> x0a -> wk -> x0b -> wv on sync: the first matmul waited
            # ~20us). sync: weights in use-order; scalar: both x halves.
            # GpSimd: biases, E(qc0), the V'-ones memset, then wo.
            # Startup DMA choreography. A contiguous descriptor runs on ONE
            # DMA engine (~37GB/s), so parallelism comes from descriptor
            # count; each dma_start costs ~0.6us of issue time on its
            # queue, and engines serve descriptors roughly FIFO. So: issue
            # the critical tensors (wq single-chunk x8, x-tokens-0..511)
            # first, then wk/x0b (needed a few us later), and demote E/wo
            # (needed at ~45us/~200us) to after wv. Tile's AP-overlap
            # tracking lets the first matmul start once wq chunk 0 + x
            # chunk 0 land.
            def stage_w(eng, dst_sb, src, n_chunks, step):
                src3 = src.rearrange("p (c m) -> p c m", c=n_chunks)
                for i in range(0, n_chunks, step):
                    eng.dma_start(dst_sb[:, i:i + step, :],
                                  src3[:, i:i + step, :])

            # PE warmup: the HAM clock gate keeps the array at 1.2GHz until
            # ~3.4us of sustained matmul activity. Run dummy matmuls on a
            # memset tile while the startup DMAs stream so the real
            # projection matmuls start at 2.4GHz.
            warm_sb = persist.tile([P, QC], BF16)
            nc.vector.memset(warm_sb[:], 0)
            for i in range(16):
                wps = psO.tile([P, QC], F32, tag="o")
                nc.tensor.matmul(wps[:], warm_sb[:, 0:P], warm_sb[:],
                                 start=True, stop=True)

            xts0 = stage_xts(0, split=True)     # x tokens 0..511 on scalar
            stage_w(nc.sync, wq_sb, wq_t, NCC, 1)
            # wk single-chunk descriptors split across both queues: the
            # first K-projection group contracts ALL 8 chunks at ~15us in
            wk3 = wk_t.rearrange("p (c m) -> p c m", c=NCC)
            for i in range(NCC):
                eng = nc.scalar if i % 2 else nc.sync
                eng.dma_start(wk_sb[:, i:i + 1, :], wk3[:, i:i + 1, :])
            nc.scalar.dma_start(xts0[:, :, QC:], x_t3[:, :, QC:QW])
            stage_w(nc.sync, wv_sb, wv_t, NCC, 2)
            nc.gpsimd.dma_start(bq_sb[:], bq[:])
            nc.gpsimd.dma_start(bk_sb[:], bk[:])
            nc.gpsimd.dma_start(bv_sb[:], bv[:])

            # ------------- generators (emit units, yield cycle costs) ------

            def g_proj(qp, half, xts):
                """Projection of one 512-token half-window: Q, K (4 dim-
                groups each) and V (4 token-groups). ~4k PE cycles/unit."""
                hq = slice(half * QC, (half + 1) * QC)
                hw_ = slice(qp * QW + half * QC, qp * QW + (half + 1) * QC)
                for kind in ("q", "k"):
                    w_sb = wq_sb if kind == "q" else wk_sb
                    b_sb = bq_sb if kind == "q" else bk_sb
                    dst = qt_buf if kind == "q" else kt_buf
                    for qd in range(NQD):
                        ps = psA.tile([P, QC], F32, tag="psA")
                        for cc in range(NCC):
                            nc.tensor.matmul(
                                ps[:],
                                w_sb[:, cc, qd * P:(qd + 1) * P],
                                xts[:, cc, hq],
                                start=(cc == 0), stop=(cc == NCC - 1),
                            )
                            # per-MM yields: fine-grained units let the
                            # interleaver slot 1-2 matmuls into small
                            # dependency gaps of the attention stream
                            # instead of displacing it by a whole group
                            yield QC
                        nc.vector.tensor_tensor(
                            dst[:, qd, hw_], ps[:],
                            b_sb[:, qd:qd + 1].to_broadcast([P, QC]),
                            mybir.AluOpType.add,
                        )
                for ts_ in range(half * (QW // P // 2),
                                 (half + 1) * (QW // P // 2)):
                    kc_idx = qp * (QW // P) + ts_
                    ps = psA.tile([P, QC], F32, tag="psA")
                    for cc in range(NCC):
                        nc.tensor.matmul(
                            ps[:],
                            xts[:, cc, ts_ * P:(ts_ + 1) * P],
                            wv_sb[:, cc, :],
                            start=(cc == 0), stop=(cc == NCC - 1),
                        )
                        yield QC
                    nc.vector.tensor_tensor(
                        vp_buf[:, kc_idx, :, 0:HD],
                        ps[:].rearrange("p (h d) -> p h d", d=HD),
                        bv_sb.rearrange("p (h d) -> p h d", d=HD),
                        mybir.AluOpType.add,
                    )

            def g_attn(qc, tail_out=None):
                """Attention for one 512-query chunk, all 8 heads processed
                as 4 head-PAIRS: head 2i occupies PE rows 0-63, head 2i+1
                rows 64-127. Per kc-block, BOTH heads' scores go into ONE
                2-bank PSUM tile (h0 at cols 0:c = bank 0, h1 at cols
                QC:QC+c = bank 1): the two matmuls write different banks so
                the PE runs them concurrently (measured 2x), and the single
                exp covering both halves means the next round's pair waits
                on ONE semaphore that releases both matmuls together --
                per-head exps would serialize the pair again (measured).
                attn@v keeps a small lag; normalize via DVE reciprocal."""
                pairs = _qc_pairs(qc)
                n_pair = len(pairs)
                e_sb = e_tiles[qc]
                qbase = qc * QC
                ot_buf = otile.tile([P, NQD, QC], BF16, tag="ot")

                if qc + 1 < N_QC:
                    stage_e(qc + 1)

                av_queue = []   # deferred (cost, closure) attn@v emissions
                n_kc = 4 * (qc + 1)

                def emit_av(o_ps, h, blocks):
                    for (kc, p_sb, poff, cols) in blocks:
                        nc.tensor.matmul(
                            o_ps[:, QC - cols:QC],
                            vp_buf[:, kc, h, :],
                            p_sb[:, poff:poff + cols],
                            start=(kc == 0), stop=(kc == n_kc - 1),
                        )

                def norm_ops(o_ps, h):
                    """1/d: bounce the denominator rows out of PSUM on
                    ScalarE (custom DVE ops misbehave on PSUM / mismatched
                    base partitions), reciprocal_approx_fast (~18 bits) on
                    DVE from SBUF, then one DVE multiply."""
                    hp = (h % 2) * HD
                    hc = h // 2
                    d_sb = wkn.tile([HD, QC], F32, tag="dsb")
                    rinv = wkn.tile([HD, QC], F32, tag="rinv")
                    nc.vector.tensor_copy(d_sb[:], o_ps[HD:2 * HD, :])
                    nc.vector.reciprocal_approx_fast(rinv[:], d_sb[:])
                    nc.vector.tensor_tensor(
                        ot_buf[hp:hp + HD, hc, :], o_ps[0:HD, :],
                        rinv[:], mybir.AluOpType.mult)

                for hp_i in range(HPC // 2):
                    h0 = 2 * hp_i            # rows 0-63, dim-chunk hp_i
                    h1 = 2 * hp_i + 1        # rows 64-127, same chunk
                    o_ps0 = psO.tile([P, QC], F32, tag="o")
                    o_ps1 = psO.tile([P, QC], F32, tag="o")
                    for kp in range(n_pair):
                        blk = pairs[kp]
                        ctot = sum(c for (_, _, c) in blk)
                        # drain lagged attn@v BEFORE this kp's score MMs:
                        # the first score MM waits on the previous kp's exp
                        # (single-buffered PSUM), and the av matmuls sitting
                        # ahead of it in the queue keep the PE busy through
                        # that wait (behind it they'd be head-of-line
                        # blocked).
                        while len(av_queue) > 2:
                            c, fn = av_queue.pop(0)
                            fn()
                            yield ctot + c
                        binfo = []
                        for (kc, j, cols) in blk:
                            s_t = psS.tile([P, 2 * QC], F32, tag="s")
                            ksl = slice(kc * KC, (kc + 1) * KC)
                            qsl = slice(qbase + j * KC, qbase + QC)
                            nc.tensor.matmul(
                                s_t[:, 0:cols],
                                kt_buf[0:HD, hp_i, ksl],
                                qt_buf[0:HD, hp_i, qsl],
                                start=True, stop=True,
                            )
                            nc.tensor.matmul(
                                s_t[:, QC:QC + cols],
                                kt_buf[HD:P, hp_i, ksl],
                                qt_buf[HD:P, hp_i, qsl],
                                start=True, stop=True,
                            )
                            binfo.append((kc, s_t, cols))
                        eoff = e_offsets[qc][kp]
                        blocks0 = []
                        blocks1 = []
                        for (kc, s_t, cols) in binfo:
                            p_t = wkp.tile([P, 2 * QC], BF16, tag="p")
                            sv = s_t.rearrange(
                                "p (b c) -> p b c", b=2)[:, :, 0:cols]
                            pv = p_t.rearrange(
                                "p (b c) -> p b c", b=2)[:, :, 0:cols]
                            nc.scalar.activation(
                                pv, sv, mybir.ActivationFunctionType.Exp)
                            ev = e_sb[:, eoff:eoff + cols].rearrange(
                                "p (b c) -> p b c", b=1).to_broadcast(
                                [P, 2, cols])
                            nc.vector.tensor_tensor(
                                pv, pv, ev, mybir.AluOpType.mult)
                            blocks0.append((kc, p_t, 0, cols))
                            blocks1.append((kc, p_t, QC, cols))
                            eoff += cols
                        av_queue.append(
                            (ctot, (lambda o=o_ps0, hh=h0, bb=blocks0:
                                    emit_av(o, hh, bb))))
                        av_queue.append(
                            (ctot, (lambda o=o_ps1, hh=h1, bb=blocks1:
                                    emit_av(o, hh, bb))))
                    # pair ends: flush this pair's remaining av matmuls and
                    # normalize both heads (frees the o_ps banks before the
                    # next pair's attn@v needs them; next pair's SCORE
                    # matmuls don't wait on this). For the LAST pair the
                    # caller may defer this tail: the final avs wait on the
                    # exp->mult chain of the last kp, so the caller emits
                    # some ready PE work ahead of them.
                    if tail_out is not None and hp_i == HPC // 2 - 1:
                        def tail_fn(q=list(av_queue), o0=o_ps0, o1=o_ps1,
                                    hh0=h0, hh1=h1):
                            for c, fn in q:
                                fn()
                            norm_ops(o0, hh0)
                            norm_ops(o1, hh1)
                        av_queue.clear()
                        tail_out.append(tail_fn)
                    else:
                        while av_queue:
                            c, fn = av_queue.pop(0)
                            fn()
                            yield c
                        norm_ops(o_ps0, h0)
                        norm_ops(o_ps1, h1)
                return_tiles[qc] = ot_buf

            e_offsets = []
            for qc in range(N_QC):
                offs = []
                off = 0
                for pr in _qc_pairs(qc):
                    offs.append(off)
                    off += sum(c for (_, _, c) in pr)
                e_offsets.append(offs)

            return_tiles = {}

            def g_outproj(qc):
                ot_buf = return_tiles[qc]
                for m in range(QC // P):
                    for half in range(2):
                        hn = slice(half * QC, (half + 1) * QC)
                        ps = psA.tile([P, QC], F32, tag="psA")
                        for cc in range(NQD):
                            nc.tensor.matmul(
                                ps[:],
                                ot_buf[:, cc, m * P:(m + 1) * P],
                                wo_sb[:, cc, hn],
                                start=(cc == 0), stop=(cc == NQD - 1),
                            )
                            yield QC
                        o_sb = wko.tile([P, QC], BF16, tag="osb")
                        # PSUM->SBUF cast on DVE (GpSimd cannot read PSUM)
                        blk = ((qc * 4 + m) * 2 + half) * P
                        ysl = y[blk:blk + P, :]
                        if qc == N_QC - 1 and m >= QC // P - 2:
                            # kernel tail: split cast+DMA in half and issue
                            # on two queues so the last bytes leave sooner
                            nc.vector.tensor_copy(o_sb[:, 0:QC // 2],
                                                  ps[:, 0:QC // 2])
                            nc.scalar.dma_start(
                                ysl[:, 0:QC // 2], o_sb[:, 0:QC // 2])
                            nc.vector.tensor_copy(o_sb[:, QC // 2:],
                                                  ps[:, QC // 2:])
                            nc.sync.dma_start(
                                ysl[:, QC // 2:], o_sb[:, QC // 2:])
                        else:
                            nc.vector.tensor_copy(o_sb[:], ps[:])
                            nc.sync.dma_start(ysl, o_sb[:])

            def interleave(*gens):
                """Proportional-fair emit: drain all generators, advancing
                the one with the lowest emitted-cycles fraction. Generators
                are (gen, total_cycles[, start_credit]) tuples; a credit
                delays a stream's first emissions (PE executes in-order, so
                a stream whose inputs arrive late must not lead the queue)."""
                state = [[g[0], g[1], 0.0 + (g[2] if len(g) > 2 else 0)]
                         for g in gens]
                while state:
                    state.sort(key=lambda s: s[2] / s[1])
                    s = state[0]
                    try:
                        c = next(s[0])
                        s[2] += c
                    except StopIteration:
                        state.remove(s)

            def run(gen):
                for _ in gen:
                    pass

            def chain(*gens):
                for g in gens:
                    yield from g

            # per-qc attention PE cost: paired scores (cols) + attn@v
            # (2*cols) per head-pair, 4 pairs
            attn_cycles = [12 * _qc_cols(qc) for qc in range(N_QC)]
            proj_half = 12 * NCC * QC
            outp_cycles = 8 * NQD * QC

            # E(qc0) and wo on the sync queue BEHIND wq/wv: engines serve
            # them only after the critical phase-0 tensors
            stage_e(0, eng=nc.sync)
            nc.gpsimd.memset(vp_buf[:, :, :, HD:], 1.0)
            stage_w(nc.sync, wo_sb, wo_t, NQD, 1)
            # phase 0: first half-window projections (nothing to overlap)
            run(g_proj(0, 0, xts0))
            # phase 1: attention(0) + second half of window-0 projections
            interleave((g_attn(0), attn_cycles[0]),
                       (g_proj(0, 1, xts0), proj_half))
            xts1 = stage_xts(1)
            # phase 2: attention(1) + window-1 first half. The proj stream
            # gets a start credit: its x tiles are still in flight when the
            # phase begins.
            interleave((g_attn(1), attn_cycles[1]),
                       (g_proj(1, 0, xts1), proj_half, 8000))
            # phase 3: attention(2) + window-1 second half
            interleave((g_attn(2), attn_cycles[2]),
                       (g_proj(1, 1, xts1), proj_half))
            # phase 4: attention(3) + out-proj(0,1,2) — ScalarE is the hot
            # engine during qc3, so park all deferrable PE work here. Hold
            # the last 4 out-proj units out of the interleave and emit them
            # between attn(3)'s main stream and its deferred tail: they're
            # dependency-free PE work that covers the final exp->mult->av
            # latency bubble.
            def take(gen, n):
                for i, c in enumerate(gen):
                    yield c
                    if i + 1 >= n:
                        return

            op012 = chain(g_outproj(0), g_outproj(1), g_outproj(2))
            a3_tail = []
            # 96 per-MM units total; hold the last 16 (4 groups) back
            interleave((g_attn(3, tail_out=a3_tail), attn_cycles[3]),
                       (take(op012, 80), 80 * QC))
            run(op012)          # remaining 4 out-proj units
            a3_tail[0]()        # last avs + normalizes
            run(g_outproj(3))

    nc.finalize()
    return nc


_PROGRAM = None


def _get_program():
    global _PROGRAM
    if _PROGRAM is None:
        _PROGRAM = build_program()
    return _PROGRAM


def _bf16(a):
    import ml_dtypes
    return np.ascontiguousarray(np.asarray(a, np.float32)).astype(
        ml_dtypes.bfloat16)


def _pack_e(dna_bias):
    """Host-packed E = (exp(bias)*causal).T in the ragged per-(qc, pair)
    column layout the kernel consumes."""
    bias = np.asarray(dna_bias, np.float32)[:T, :T]
    causal = np.tril(np.ones((T, T), np.float32))
    e_t = (np.exp(bias) * causal).T  # [keys, queries]
    out = np.zeros((P, N_QC * E_STRIDE), np.float32)
    for qc in range(N_QC):
        off = 0
        for pr in _qc_pairs(qc):
            for (kc, j, cols) in pr:
                blk = e_t[kc * KC:(kc + 1) * KC,
                          qc * QC + j * KC:(qc + 1) * QC]
                out[:, qc * E_STRIDE + off: qc * E_STRIDE + off + cols] = blk
                off += cols
    return _bf16(out)


def make_in_maps(x, qkv_w, qkv_b, out_w, out_b, dna_bias):
    x = np.asarray(x, np.float32)
    qkv_w = np.asarray(qkv_w, np.float32)
    qkv_b = np.asarray(qkv_b, np.float32)
    out_w = np.asarray(out_w, np.float32)

    scale = 1.0 / np.sqrt(HD)
    e_packed = _pack_e(dna_bias)

    def _swz(w_t, n_chunks):
        # [DIM_in, M] -> [128, n_chunks * M]: partition p holds chunk-major
        # rows (c*128+p) so the SBUF [P, c, M] tile loads as one DMA
        return np.ascontiguousarray(
            w_t.reshape(n_chunks, P, -1).transpose(1, 0, 2).reshape(P, -1))

    in_maps = []
    for core in range(N_CORES):
        b, g = divmod(core, 2)
        cols = slice(g * CPC, (g + 1) * CPC)
        wq = qkv_w[0 * DIM:1 * DIM][cols] * scale      # [512, 1024]
        wk = qkv_w[1 * DIM:2 * DIM][cols]
        wv = qkv_w[2 * DIM:3 * DIM][cols]
        in_maps.append({
            "x_t": _bf16(x[b].T),
            "wq_t": _bf16(_swz(wq.T, NCC)),
            "wk_t": _bf16(_swz(wk.T, NCC)),
            "wv_t": _bf16(_swz(wv.T, NCC)),
            "bq": np.ascontiguousarray(
                (qkv_b[0 * DIM:1 * DIM][cols] * scale).reshape(NQD, P).T),
            "bk": np.ascontiguousarray(
                qkv_b[1 * DIM:2 * DIM][cols].reshape(NQD, P).T),
            "bv": np.ascontiguousarray(
                np.broadcast_to(qkv_b[2 * DIM:3 * DIM][cols][None, :],
                                (P, CPC))),
            "wo_t": _bf16(_swz(out_w[:, cols].T, NQD)),
            "e_pk": e_packed,
        })
    return in_maps


LAST_RESULTS = None


def kernel(x, qkv_w, qkv_b, out_w, out_b, dna_bias, **run_kwargs):
    global LAST_RESULTS
    nc = _get_program()
    in_maps = make_in_maps(x, qkv_w, qkv_b, out_w, out_b, dna_bias)
    res = run_bass_kernel_spmd(nc, in_maps, list(range(N_CORES)), **run_kwargs)
    LAST_RESULTS = res
    out_b = np.asarray(out_b, np.float32)
    out = np.empty((B, T, DIM), np.float32)
    for b in range(B):
        yb = (np.asarray(res.results[2 * b]["y"], np.float32)
              + np.asarray(res.results[2 * b + 1]["y"], np.float32))
        # unblock [16*m-blocks, half, 128, 512] -> [T, DIM]
        out[b] = (yb.reshape(16, 2, P, QC).transpose(0, 2, 1, 3)
                  .reshape(T, DIM) + out_b)
    return out


# revision 48
# speedup vs baseline: 1.0671x; 1.0671x over previous
"""DNA-structure attention Trainium2 kernel (8-core SPMD), v3.

Reference computation (per batch b):
    qkv = x @ qkv_w.T + qkv_b ; split to q,k,v [H=16 heads, d=64]
    s   = q @ k.T / 8 + dna_bias ; causal mask ; p = softmax(s)
    o   = p @ v ; y = concat_heads(o) @ out_w.T + out_b

Sharding: 8 cores = 4 batches x 2 head-groups (8 heads each).
Each core computes its batch's partial output y_partial = o_g @ out_w[:, cols_g].T;
host sums the two partials per batch and adds out_b.

v3 changes over v2 (v2 = 300.4us -> v3 = ~269us):
  - Paired score matmuls: head 2i lives at PE rows 0-63, head 2i+1 at rows
    64-127 (the layout already guaranteed this). Emitting the two heads'
    score MMs back-to-back makes the PE run them CONCURRENTLY as 64-row
    tiles (microbenched: exactly 2.0x) -- the d=64 contraction no longer
    wastes half the array. Critical detail: both heads' scores for one
    kc-block share ONE 2-bank PSUM tile (h0 at bank 0, h1 at bank 1) so a
    SINGLE exp covers both -- with per-head tiles the next round's pair
    serializes on two different exp semaphores (measured, kills the 2x).
  - Normalization 1/d via ScalarE copy + DVE reciprocal_approx_fast
    (18-bit ok; 51-ULP) instead of ScalarE exp(-ln d): frees ~35us of
    ScalarE for the softmax exps. The custom DVE op needs same-base-
    partition SBUF operands (cross-base or PSUM input = garbage/hang).
  - attn@v matmuls drained BEFORE each kp's score MMs: the first score MM
    waits the previous kp's exp (single-buffered PSUM), and work queued
    ahead of it covers the wait (in-order PE queue head-blocks otherwise).
  - Startup: host pre-swizzles weights to the SBUF layout so each loads
    with a handful of descriptors (one contiguous descriptor = ONE DMA
    engine at ~37GB/s; splitting is what buys parallelism, but each
    dma_start costs ~0.6us of issue time on its queue -- balance). 16
    dummy matmuls pre-warm the PE HAM clock gate (1.2 -> 2.4GHz) during
    the DMA wait. First real matmul at ~11us (was ~21).
  - y written block-contiguous (host untransposes): linear 128KB writes
    instead of 128-segment strided scatters.
  - Phase-4 tail: last 4 out-proj units held out of the interleave and
    emitted between attn(3)'s main stream and its deferred final
    avs+normalizes, covering the last exp->mult->av latency bubble.
"""

import sys

if "/opt/trn_rl_repo" not in sys.path:
    sys.path.insert(0, "/opt/trn_rl_repo")

import numpy as np

import concourse.bass as bass
import concourse.mybir as mybir
import concourse.tile as tile
from concourse import bacc
from concourse.bass_utils import run_bass_kernel_spmd

# The axon NTFF-profiling hook lives in trn_agent_boot in this container but
# concourse expects it at antenv.axon_hooks (absent). Register a shim module
# so run_bass_kernel_spmd(trace=True) can capture real HW timings.
if "antenv.axon_hooks" not in sys.modules:
    import types

    def _get_axon_ntff_profile_hook(_cache=[]):
        if not _cache:
            try:
                from trn_agent_boot.trn_boot import _ntff_profile_via_ctypes
                _cache.append(
                    _ntff_profile_via_ctypes("/opt/axon/libaxon_pjrt.so"))
            except Exception:
                _cache.append(None)
        return _cache[0]

    _m = types.ModuleType("antenv.axon_hooks")
    _m.get_axon_ntff_profile_hook = _get_axon_ntff_profile_hook
    sys.modules["antenv.axon_hooks"] = _m

B, T, DIM = 4, 2048, 1024
HEADS = 16
HD = 64  # head dim
N_CORES = 8
HPC = 8            # heads per core
CPC = HPC * HD     # channel slice per core (512)
QC = 512           # query chunk
KC = 128           # key chunk
N_QC = T // QC     # 4
N_KC = T // KC     # 16
P = 128

F32 = mybir.dt.float32
BF16 = mybir.dt.bfloat16

NCC = DIM // P    # 8 contraction chunks for qkv projection
NQD = CPC // P    # 4 dim-chunks of Q/K


def _qc_pairs(qc):
    """Ragged causal plan for query chunk qc: list of pairs, each pair a
    list of (kc, j, cols) with j = 128-query offset into the chunk."""
    n_kc = 4 * (qc + 1)
    blocks = []
    for kc in range(n_kc):
        j = max(0, kc - 4 * qc)
        blocks.append((kc, j, QC - j * KC))
    return [blocks[i:i + 2] for i in range(0, n_kc, 2)]


def _qc_cols(qc):
    return sum(c for pr in _qc_pairs(qc) for (_, _, c) in pr)


E_STRIDE = _qc_cols(3)  # 7424, worst-case packed width


def build_program():
    nc = bacc.Bacc("TRN2", target_bir_lowering=False, debug=False,
                   num_devices=N_CORES)

    # weights/biases come in host-preswizzled to the SBUF layout
    # [128 partitions, chunk-major cols] so each loads as ONE DMA
    # descriptor (a "(c p) m -> p c m" rearrange on the DRAM side gets
    # split into NCC separate DIRECT2D issues at ~0.6us each, serialized
    # on the issuing sequencer -- that was ~15us of the v3 startup).
    x_t = nc.declare_dram_parameter("x_t", [DIM, T], BF16, isOutput=False)
    wq_t = nc.declare_dram_parameter("wq_t", [P, NCC * CPC], BF16,
                                     isOutput=False)
    wk_t = nc.declare_dram_parameter("wk_t", [P, NCC * CPC], BF16,
                                     isOutput=False)
    wv_t = nc.declare_dram_parameter("wv_t", [P, NCC * CPC], BF16,
                                     isOutput=False)
    bq = nc.declare_dram_parameter("bq", [P, NQD], F32, isOutput=False)
    bk = nc.declare_dram_parameter("bk", [P, NQD], F32, isOutput=False)
    bv = nc.declare_dram_parameter("bv", [P, CPC], F32, isOutput=False)
    wo_t = nc.declare_dram_parameter("wo_t", [P, NQD * DIM], BF16,
                                     isOutput=False)
    e_pk = nc.declare_dram_parameter("e_pk", [P, N_QC * E_STRIDE], BF16,
                                     isOutput=False)
    # y partials in bf16: halves the output DMA and the two partials are
    # summed in fp32 on the host (absmax error budget has 2x headroom).
    # Block-contiguous layout [qc*4+m, half, 128, 512]: each out-proj
    # result block is one LINEAR 128KB DMA write (the [T, DIM] layout made
    # every write a 128-segment strided scatter that drained ~5us after
    # the last matmul); the host untransposes.
    y = nc.declare_dram_parameter("y", [N_QC * 4 * 2 * P, QC], BF16,
                                  isOutput=True)

    with tile.TileContext(nc) as tc:
        with (
            tc.tile_pool(name="persist", bufs=1) as persist,
            tc.tile_pool(name="wts", bufs=1) as wts,
            tc.tile_pool(name="xw", bufs=2) as xw,
            tc.tile_pool(name="ebuf", bufs=1) as ebuf,
            tc.tile_pool(name="otile", bufs=4) as otile,
            tc.tile_pool(name="wkp", bufs=8) as wkp,
            tc.tile_pool(name="wko", bufs=3) as wko,
            tc.tile_pool(name="wkn", bufs=2) as wkn,
            tc.tile_pool(name="psA", bufs=2, space="PSUM") as psA,
            tc.tile_pool(name="psS", bufs=2, space="PSUM") as psS,
            tc.tile_pool(name="psO", bufs=2, space="PSUM") as psO,
        ):
            # ---------------- persistent SBUF ----------------
            qt_buf = persist.tile([P, NQD, T], BF16)   # Q.T, dims-major
            kt_buf = persist.tile([P, NQD, T], BF16)   # K.T, same layout
            # V' per (key chunk, head): cols 0:64 V, cols 64:128 ones
            vp_buf = persist.tile([P, N_KC, HPC, P], BF16)
            bq_sb = persist.tile([P, NQD], F32)
            bk_sb = persist.tile([P, NQD], F32)
            bv_sb = persist.tile([P, CPC], F32)
            wq_sb = wts.tile([P, NCC, CPC], BF16)
            wk_sb = wts.tile([P, NCC, CPC], BF16)
            wv_sb = wts.tile([P, NCC, CPC], BF16)
            wo_sb = wts.tile([P, NQD, DIM], BF16)      # out_w.T slice

            x_t3 = x_t.rearrange("(c p) t -> p c t", p=P)

            QW = 2 * QC               # projection window (1024 tokens)

            def stage_xts(qp, split=False):
                tw = slice(qp * QW, (qp + 1) * QW)
                xt = xw.tile([P, NCC, QW], BF16, tag="xt")
                if split:
                    # first 512 tokens in chunk-pair pieces: the first
                    # matmul only needs dim-chunk 0, and smaller descriptors
                    # spread over more DMA engines
                    for c in range(0, NCC, 2):
                        nc.scalar.dma_start(
                            xt[:, c:c + 2, 0:QC],
                            x_t3[:, c:c + 2, tw.start:tw.start + QC])
                else:
                    nc.sync.dma_start(xt[:], x_t3[:, :, tw])
                return xt

            # E tiles: ping-pong two buffers across qc (qc0/2 share one,
            # qc1/3 the other, sized to the larger user)
            e_tiles = {}
            E_TAG_COLS = {0: _qc_cols(2), 1: _qc_cols(3)}

            def stage_e(qc, eng=None):
                eng = eng or nc.gpsimd
                t_ = ebuf.tile([P, E_TAG_COLS[qc % 2]], BF16, tag=f"e{qc % 2}")
                e_tiles[qc] = t_
                cols = _qc_cols(qc)
                # split into <=2048-col chunks: one contiguous descriptor
                # runs on a SINGLE DMA engine (~37GB/s), so chunking is what
                # buys transfer parallelism
                off = 0
                while off < cols:
                    n = min(2048, cols - off)
                    eng.dma_start(
                        t_[:, off:off + n],
                        e_pk[:, qc * E_STRIDE + off: qc * E_STRIDE + off + n])
                    off += n

            # Startup DMA choreography: one fat DMA per tensor, spread
            # across the engine queues that can issue DMAs (sync, scalar,
            # gpsimd) so the transfers run concurrently (v2 serialized
            # wq -> x0a -> wk -> x0b -> wv on sync: the first matmul waited
            # ~20us). sync: weights in use-order; scalar: both x halves.
            # GpSimd: biases, E(qc0), the V'-ones memset, then wo.
            # Startup DMA choreography. A contiguous descriptor runs on ONE
            # DMA engine (~37GB/s), so parallelism comes from descriptor
            # count; each dma_start costs ~0.6us of issue time on its
            # queue, and engines serve descriptors roughly FIFO. So: issue
            # the critical tensors (wq single-chunk x8, x-tokens-0..511)
            # first, then wk/x0b (needed a few us later), and demote E/wo
            # (needed at ~45us/~200us) to after wv. Tile's AP-overlap
            # tracking lets the first matmul start once wq chunk 0 + x
            # chunk 0 land.
            def stage_w(eng, dst_sb, src, n_chunks, step):
                src3 = src.rearrange("p (c m) -> p c m", c=n_chunks)
                for i in range(0, n_chunks, step):
                    eng.dma_start(dst_sb[:, i:i + step, :],
                                  src3[:, i:i + step, :])

            # PE warmup: the HAM clock gate keeps the array at 1.2GHz until
            # ~3.4us of sustained matmul activity. Run dummy matmuls on a
            # memset tile while the startup DMAs stream so the real
            # projection matmuls start at 2.4GHz.
            warm_sb = persist.tile([P, QC], BF16)
            nc.vector.memset(warm_sb[:], 0)
            for i in range(16):
                wps = psO.tile([P, QC], F32, tag="o")
                nc.tensor.matmul(wps[:], warm_sb[:, 0:P], warm_sb[:],
                                 start=True, stop=True)

            xts0 = stage_xts(0, split=True)     # x tokens 0..511 on scalar
            stage_w(nc.sync, wq_sb, wq_t, NCC, 1)
            # wk single-chunk descriptors split across both queues: the
            # first K-projection group contracts ALL 8 chunks at ~15us in
            wk3 = wk_t.rearrange("p (c m) -> p c m", c=NCC)
            for i in range(NCC):
                eng = nc.scalar if i % 2 else nc.sync
                eng.dma_start(wk_sb[:, i:i + 1, :], wk3[:, i:i + 1, :])
            nc.scalar.dma_start(xts0[:, :, QC:], x_t3[:, :, QC:QW])
            stage_w(nc.sync, wv_sb, wv_t, NCC, 2)
            nc.gpsimd.dma_start(bq_sb[:], bq[:])
            nc.gpsimd.dma_start(bk_sb[:], bk[:])
            nc.gpsimd.dma_start(bv_sb[:], bv[:])

            # ------------- generators (emit units, yield cycle costs) ------

            def g_proj(qp, half, xts):
                """Projection of one 512-token half-window: Q, K (4 dim-
                groups each) and V (4 token-groups). ~4k PE cycles/unit."""
                hq = slice(half * QC, (half + 1) * QC)
                hw_ = slice(qp * QW + half * QC, qp * QW + (half + 1) * QC)
                for kind in ("q", "k"):
                    w_sb = wq_sb if kind == "q" else wk_sb
                    b_sb = bq_sb if kind == "q" else bk_sb
                    dst = qt_buf if kind == "q" else kt_buf
                    for qd in range(NQD):
                        ps = psA.tile([P, QC], F32, tag="psA")
                        for cc in range(NCC):
                            nc.tensor.matmul(
                                ps[:],
                                w_sb[:, cc, qd * P:(qd + 1) * P],
                                xts[:, cc, hq],
                                start=(cc == 0), stop=(cc == NCC - 1),
                            )
                            # per-MM yields: fine-grained units let the
                            # interleaver slot 1-2 matmuls into small
                            # dependency gaps of the attention stream
                            # instead of displacing it by a whole group
                            yield QC
                        nc.vector.tensor_tensor(
                            dst[:, qd, hw_], ps[:],
                            b_sb[:, qd:qd + 1].to_broadcast([P, QC]),
                            mybir.AluOpType.add,
                        )
                for ts_ in range(half * (QW // P // 2),
                                 (half + 1) * (QW // P // 2)):
                    kc_idx = qp * (QW // P) + ts_
                    ps = psA.tile([P, QC], F32, tag="psA")
                    for cc in range(NCC):
                        nc.tensor.matmul(
                            ps[:],
                            xts[:, cc, ts_ * P:(ts_ + 1) * P],
                            wv_sb[:, cc, :],
                            start=(cc == 0), stop=(cc == NCC - 1),
                        )
                        yield QC
                    nc.vector.tensor_tensor(
                        vp_buf[:, kc_idx, :, 0:HD],
                        ps[:].rearrange("p (h d) -> p h d", d=HD),
                        bv_sb.rearrange("p (h d) -> p h d", d=HD),
                        mybir.AluOpType.add,
                    )

            def g_attn(qc, tail_out=None):
                """Attention for one 512-query chunk, all 8 heads processed
                as 4 head-PAIRS: head 2i occupies PE rows 0-63, head 2i+1
                rows 64-127. Per kc-block, BOTH heads' scores go into ONE
                2-bank PSUM tile (h0 at cols 0:c = bank 0, h1 at cols
                QC:QC+c = bank 1): the two matmuls write different banks so
                the PE runs them concurrently (measured 2x), and the single
                exp covering both halves means the next round's pair waits
                on ONE semaphore that releases both matmuls together --
                per-head exps would serialize the pair again (measured).
                attn@v keeps a small lag; normalize via DVE reciprocal."""
                pairs = _qc_pairs(qc)
                n_pair = len(pairs)
                e_sb = e_tiles[qc]
                qbase = qc * QC
                ot_buf = otile.tile([P, NQD, QC], BF16, tag="ot")

                if qc + 1 < N_QC:
                    stage_e(qc + 1)

                av_queue = []   # deferred (cost, closure) attn@v emissions
                n_kc = 4 * (qc + 1)

                def emit_av(o_ps, h, blocks):
                    for (kc, p_sb, poff, cols) in blocks:
                        nc.tensor.matmul(
                            o_ps[:, QC - cols:QC],
                            vp_buf[:, kc, h, :],
                            p_sb[:, poff:poff + cols],
                            start=(kc == 0), stop=(kc == n_kc - 1),
                        )

                def norm_ops(o_ps, h):
                    """1/d: bounce the denominator rows out of PSUM on
                    ScalarE (custom DVE ops misbehave on PSUM / mismatched
                    base partitions), reciprocal_approx_fast (~18 bits) on
                    DVE from SBUF, then one DVE multiply."""
                    hp = (h % 2) * HD
                    hc = h // 2
                    d_sb = wkn.tile([HD, QC], F32, tag="dsb")
                    rinv = wkn.tile([HD, QC], F32, tag="rinv")
                    nc.scalar.copy(d_sb[:], o_ps[HD:2 * HD, :])
                    nc.vector.reciprocal_approx_fast(rinv[:], d_sb[:])
                    nc.vector.tensor_tensor(
                        ot_buf[hp:hp + HD, hc, :], o_ps[0:HD, :],
                        rinv[:], mybir.AluOpType.mult)

                for hp_i in range(HPC // 2):
                    h0 = 2 * hp_i            # rows 0-63, dim-chunk hp_i
                    h1 = 2 * hp_i + 1        # rows 64-127, same chunk
                    o_ps0 = psO.tile([P, QC], F32, tag="o")
                    o_ps1 = psO.tile([P, QC], F32, tag="o")
                    for kp in range(n_pair):
                        blk = pairs[kp]
                        ctot = sum(c for (_, _, c) in blk)
                        # drain lagged attn@v BEFORE this kp's score MMs:
                        # the first score MM waits on the previous kp's exp
                        # (single-buffered PSUM), and the av matmuls sitting
                        # ahead of it in the queue keep the PE busy through
                        # that wait (behind it they'd be head-of-line
                        # blocked).
                        while len(av_queue) > 2:
                            c, fn = av_queue.pop(0)
                            fn()
                            yield ctot + c
                        binfo = []
                        for (kc, j, cols) in blk:
                            s_t = psS.tile([P, 2 * QC], F32, tag="s")
                            ksl = slice(kc * KC, (kc + 1) * KC)
                            qsl = slice(qbase + j * KC, qbase + QC)
                            nc.tensor.matmul(
                                s_t[:, 0:cols],
                                kt_buf[0:HD, hp_i, ksl],
                                qt_buf[0:HD, hp_i, qsl],
                                start=True, stop=True,
                            )
                            nc.tensor.matmul(
                                s_t[:, QC:QC + cols],
                                kt_buf[HD:P, hp_i, ksl],
                                qt_buf[HD:P, hp_i, qsl],
                                start=True, stop=True,
                            )
                            binfo.append((kc, s_t, cols))
                        eoff = e_offsets[qc][kp]
                        blocks0 = []
                        blocks1 = []
                        for (kc, s_t, cols) in binfo:
                            p_t = wkp.tile([P, 2 * QC], BF16, tag="p")
                            sv = s_t.rearrange(
                                "p (b c) -> p b c", b=2)[:, :, 0:cols]
                            pv = p_t.rearrange(
                                "p (b c) -> p b c", b=2)[:, :, 0:cols]
                            nc.scalar.activation(
                                pv, sv, mybir.ActivationFunctionType.Exp)
                            ev = e_sb[:, eoff:eoff + cols].rearrange(
                                "p (b c) -> p b c", b=1).to_broadcast(
                                [P, 2, cols])
                            nc.vector.tensor_tensor(
                                pv, pv, ev, mybir.AluOpType.mult)
                            blocks0.append((kc, p_t, 0, cols))
                            blocks1.append((kc, p_t, QC, cols))
                            eoff += cols
                        av_queue.append(
                            (ctot, (lambda o=o_ps0, hh=h0, bb=blocks0:
                                    emit_av(o, hh, bb))))
                        av_queue.append(
                            (ctot, (lambda o=o_ps1, hh=h1, bb=blocks1:
                                    emit_av(o, hh, bb))))
                    # pair ends: flush this pair's remaining av matmuls and
                    # normalize both heads (frees the o_ps banks before the
                    # next pair's attn@v needs them; next pair's SCORE
                    # matmuls don't wait on this). For the LAST pair the
                    # caller may defer this tail: the final avs wait on the
                    # exp->mult chain of the last kp, so the caller emits
                    # some ready PE work ahead of them.
                    if tail_out is not None and hp_i == HPC // 2 - 1:
                        def tail_fn(q=list(av_queue), o0=o_ps0, o1=o_ps1,
                                    hh0=h0, hh1=h1):
                            for c, fn in q:
                                fn()
                            norm_ops(o0, hh0)
                            norm_ops(o1, hh1)
                        av_queue.clear()
                        tail_out.append(tail_fn)
                    else:
                        while av_queue:
                            c, fn = av_queue.pop(0)
                            fn()
                            yield c
                        norm_ops(o_ps0, h0)
                        norm_ops(o_ps1, h1)
                return_tiles[qc] = ot_buf

            e_offsets = []
            for qc in range(N_QC):
                offs = []
                off = 0
                for pr in _qc_pairs(qc):
                    offs.append(off)
                    off += sum(c for (_, _, c) in pr)
                e_offsets.append(offs)

            return_tiles = {}

            def g_outproj(qc):
                ot_buf = return_tiles[qc]
                for m in range(QC // P):
                    for half in range(2):
                        hn = slice(half * QC, (half + 1) * QC)
                        ps = psA.tile([P, QC], F32, tag="psA")
                        for cc in range(NQD):
                            nc.tensor.matmul(
                                ps[:],
                                ot_buf[:, cc, m * P:(m + 1) * P],
                                wo_sb[:, cc, hn],
                                start=(cc == 0), stop=(cc == NQD - 1),
                            )
                            yield QC
                        o_sb = wko.tile([P, QC], BF16, tag="osb")
                        # PSUM->SBUF cast on DVE (GpSimd cannot read PSUM)
                        blk = ((qc * 4 + m) * 2 + half) * P
                        ysl = y[blk:blk + P, :]
                        if qc == N_QC - 1 and m >= QC // P - 2:
                            # kernel tail: split cast+DMA in half and issue
                            # on two queues so the last bytes leave sooner
                            nc.vector.tensor_copy(o_sb[:, 0:QC // 2],
                                                  ps[:, 0:QC // 2])
                            nc.scalar.dma_start(
                                ysl[:, 0:QC // 2], o_sb[:, 0:QC // 2])
                            nc.vector.tensor_copy(o_sb[:, QC // 2:],
                                                  ps[:, QC // 2:])
                            nc.sync.dma_start(
                                ysl[:, QC // 2:], o_sb[:, QC // 2:])
                        else:
                            nc.vector.tensor_copy(o_sb[:], ps[:])
                            nc.sync.dma_start(ysl, o_sb[:])

            def interleave(*gens):
                """Proportional-fair emit: drain all generators, advancing
                the one with the lowest emitted-cycles fraction. Generators
                are (gen, total_cycles[, start_credit]) tuples; a credit
                delays a stream's first emissions (PE executes in-order, so
                a stream whose inputs arrive late must not lead the queue)."""
                state = [[g[0], g[1], 0.0 + (g[2] if len(g) > 2 else 0)]
                         for g in gens]
                while state:
                    state.sort(key=lambda s: s[2] / s[1])
                    s = state[0]
                    try:
                        c = next(s[0])
                        s[2] += c
                    except StopIteration:
                        state.remove(s)

            def run(gen):
                for _ in gen:
                    pass

            def chain(*gens):
                for g in gens:
                    yield from g

            # per-qc attention PE cost: paired scores (cols) + attn@v
            # (2*cols) per head-pair, 4 pairs
            attn_cycles = [12 * _qc_cols(qc) for qc in range(N_QC)]
            proj_half = 12 * NCC * QC
            outp_cycles = 8 * NQD * QC

            # E(qc0) and wo on the sync queue BEHIND wq/wv: engines serve
            # them only after the critical phase-0 tensors
            stage_e(0, eng=nc.sync)
            nc.gpsimd.memset(vp_buf[:, :, :, HD:], 1.0)
            stage_w(nc.sync, wo_sb, wo_t, NQD, 1)
            # phase 0: first half-window projections (nothing to overlap)
            run(g_proj(0, 0, xts0))
            # phase 1: attention(0) + second half of window-0 projections
            interleave((g_attn(0), attn_cycles[0]),
                       (g_proj(0, 1, xts0), proj_half))
            xts1 = stage_xts(1)
            # phase 2: attention(1) + window-1 first half. The proj stream
            # gets a start credit: its x tiles are still in flight when the
            # phase begins.
            interleave((g_attn(1), attn_cycles[1]),
                       (g_proj(1, 0, xts1), proj_half, 8000))
            # phase 3: attention(2) + window-1 second half
            interleave((g_attn(2), attn_cycles[2]),
                       (g_proj(1, 1, xts1), proj_half))
            # phase 4: attention(3) + out-proj(0,1,2) — ScalarE is the hot
            # engine during qc3, so park all deferrable PE work here. Hold
            # the last 4 out-proj units out of the interleave and emit them
            # between attn(3)'s main stream and its deferred tail: they're
            # dependency-free PE work that covers the final exp->mult->av
            # latency bubble.
            def take(gen, n):
                for i, c in enumerate(gen):
                    yield c
                    if i + 1 >= n:
                        return

            op012 = chain(g_outproj(0), g_outproj(1), g_outproj(2))
            a3_tail = []
            # 96 per-MM units total; hold the last 16 (4 groups) back
            interleave((g_attn(3, tail_out=a3_tail), attn_cycles[3]),
                       (take(op012, 80), 80 * QC))
            run(op012)          # remaining 4 out-proj units
            a3_tail[0]()        # last avs + normalizes
            run(g_outproj(3))

    nc.finalize()
    return nc


_PROGRAM = None


def _get_program():
    global _PROGRAM
    if _PROGRAM is None:
        _PROGRAM = build_program()
    return _PROGRAM


def _bf16(a):
    import ml_dtypes
    return np.ascontiguousarray(np.asarray(a, np.float32)).astype(
        ml_dtypes.bfloat16)


def _pack_e(dna_bias):
    """Host-packed E = (exp(bias)*causal).T in the ragged per-(qc, pair)
    column layout the kernel consumes."""
    bias = np.asarray(dna_bias, np.float32)[:T, :T]
    causal = np.tril(np.ones((T, T), np.float32))
    e_t = (np.exp(bias) * causal).T  # [keys, queries]
    out = np.zeros((P, N_QC * E_STRIDE), np.float32)
    for qc in range(N_QC):
        off = 0
        for pr in _qc_pairs(qc):
            for (kc, j, cols) in pr:
                blk = e_t[kc * KC:(kc + 1) * KC,
                          qc * QC + j * KC:(qc + 1) * QC]
                out[:, qc * E_STRIDE + off: qc * E_STRIDE + off + cols] = blk
                off += cols
    return _bf16(out)


def make_in_maps(x, qkv_w, qkv_b, out_w, out_b, dna_bias):
    x = np.asarray(x, np.float32)
    qkv_w = np.asarray(qkv_w, np.float32)
    qkv_b = np.asarray(qkv_b, np.float32)
    out_w = np.asarray(out_w, np.float32)

    scale = 1.0 / np.sqrt(HD)
    e_packed = _pack_e(dna_bias)

    def _swz(w_t, n_chunks):
        # [DIM_in, M] -> [128, n_chunks * M]: partition p holds chunk-major
        # rows (c*128+p) so the SBUF [P, c, M] tile loads as one DMA
        return np.ascontiguousarray(
            w_t.reshape(n_chunks, P, -1).transpose(1, 0, 2).reshape(P, -1))

    in_maps = []
    for core in range(N_CORES):
        b, g = divmod(core, 2)
        cols = slice(g * CPC, (g + 1) * CPC)
        wq = qkv_w[0 * DIM:1 * DIM][cols] * scale      # [512, 1024]
        wk = qkv_w[1 * DIM:2 * DIM][cols]
        wv = qkv_w[2 * DIM:3 * DIM][cols]
        in_maps.append({
            "x_t": _bf16(x[b].T),
            "wq_t": _bf16(_swz(wq.T, NCC)),
            "wk_t": _bf16(_swz(wk.T, NCC)),
            "wv_t": _bf16(_swz(wv.T, NCC)),
            "bq": np.ascontiguousarray(
                (qkv_b[0 * DIM:1 * DIM][cols] * scale).reshape(NQD, P).T),
            "bk": np.ascontiguousarray(
                qkv_b[1 * DIM:2 * DIM][cols].reshape(NQD, P).T),
            "bv": np.ascontiguousarray(
                np.broadcast_to(qkv_b[2 * DIM:3 * DIM][cols][None, :],
                                (P, CPC))),
            "wo_t": _bf16(_swz(out_w[:, cols].T, NQD)),
            "e_pk": e_packed,
        })
    return in_maps


LAST_RESULTS = None


def kernel(x, qkv_w, qkv_b, out_w, out_b, dna_bias, **run_kwargs):
    global LAST_RESULTS
    nc = _get_program()
    in_maps = make_in_maps(x, qkv_w, qkv_b, out_w, out_b, dna_bias)
    res = run_bass_kernel_spmd(nc, in_maps, list(range(N_CORES)), **run_kwargs)
    LAST_RESULTS = res
    out_b = np.asarray(out_b, np.float32)
    out = np.empty((B, T, DIM), np.float32)
    for b in range(B):
        yb = (np.asarray(res.results[2 * b]["y"], np.float32)
              + np.asarray(res.results[2 * b + 1]["y"], np.float32))
        # unblock [16*m-blocks, half, 128, 512] -> [T, DIM]
        out[b] = (yb.reshape(16, 2, P, QC).transpose(0, 2, 1, 3)
                  .reshape(T, DIM) + out_b)
    return out
